# revision 31
# baseline (speedup 1.0000x reference)
"""Trainium2 Bass kernel for the water-network leak MSE model.

Math (reference):
    net(s)   = base[idx_s] + MLP(idx_s)                    (idx_s in [0,1024))
    q        = D @ inv_ev + net*PM[:, idx]                 (PM = inv^T M)
    hL       = K * q|q|^0.852,  K = 10.667 C^-1.852 d^-4.871 L
    H        = hsup - hL @ invp^T,  hsup = invp @ supply
    d_leak   = c0 * Mp[:, idx] * sqrt(relu(H)),  c0 = Cd*a*sqrt(2g)
    res      = D @ A0invF^T + net*AM[:, idx] - d_leak
    out      = mean(res^2)

Device strategy (8 cores data-parallel, 2048 samples/core, chunks
[256,512,512,512,256], software-pipelined A/B/CE stages 3 deep):
  The Hazen-Williams K is folded per-pipe into the H-matmul weights so q
  carries no per-pipe scale; everything feeding only d_leak tolerates
  fp8. The q matmuls (D8 @ inv_ev8, K=256) and the residual D-part
  (D8 @ A0invF8) run as fp8-e4m3 DoubleRow matmuls (2 K-blocks per
  instruction, ~1.4x PE throughput; D ships once, as fp8). H matmuls
  use fp8 weights with the bf16 hl stream (DR fp8-hl variants lose more
  on the DVE fp8 conversion, which drops to 1x rate, than DR wins on
  PE). hsup is PE-injected into the H PSUM via a K=1 matmul so relu
  needs no per-bank bias and runs on 2-bank pairs. d_leak is computed
  NEGATED (table -c0*Mp*g2, bit-trick sqrt (i>>1)+0x1FC0) and
  PE-injected into the residual PSUM through the same jv*identity used
  for the AM gather rows, so Square reads the residual straight from
  PSUM with a descale factor and accumulates per-chunk partial sums -
  no residual drain op at all. The q drain is a fused
  scalar_tensor_tensor (PSUM descale + fp8 gather-row add).
  Per-sample gather rows (PM*table fp8, c0*Mp bf16, AM*table fp8) are
  gathered on host; per chunk they ship as two tensors (q-side chA /
  M-side chB, needed 2 steps apart) and all weights as one packed
  tensor: 13 DMA triggers total (each costs ~0.7us serialized on the
  Sync engine - 19 triggers dominated the old 14us head). PSUM: qp
  [P,2,CH]x2 ring (stage A) + hp [P,2,CH]x2 ring shared by the H and
  residual accumulators (alloc order hp0,hp1,rp0,rp1 keeps reuse one
  step apart) = 8 banks. Tiny N=64 warmup matmuls pad the fill-phase
  bursts against the HAM half-clock gate.
  Measured: ~87us (baseline 87.2us), rel err 7.7e-4 (gate 2e-2),
  validated bit-exactly against a numpy emulation of every
  quantization/bit-trick choice.
"""

import math

import numpy as np
import ml_dtypes

P = 128
N_CORES = 8
S_TOTAL = 16384
SC = S_TOTAL // N_CORES
CS = [256, 512, 512, 512, 256]   # per-chunk sample counts
NCH = len(CS)
N_NODES = 512
N_PIPES = 1024
N_DEM = 256
G_ACC = 9.80665
SIG = 1
SD = 2.0 ** 7

BF16 = ml_dtypes.bfloat16
E4M3 = ml_dtypes.float8_e4m3fn

# wrest packing offsets (bytes per partition)
OFF_INVPT = 0          # [4,4,2,128] fp8        -> 4096
OFF_A0INV = 4096       # [4,2,128] fp8          -> 1024
OFF_IDENT = 5120       # [128] bf16             -> 256
OFF_HSUPW = 5376       # [512] bf16             -> 1024
OFF_LNB = 6400         # [1] f32                -> 4
WREST = 6416

_MODULE_CACHE: dict = {}


def _build_module(scal):
    import concourse.bacc as bacc
    import concourse.mybir as mybir
    import concourse.tile as tile

    f32 = mybir.dt.float32
    bf16 = mybir.dt.bfloat16
    fp8 = mybir.dt.float8e4
    u8 = mybir.dt.uint8
    i16 = mybir.dt.int16
    AF = mybir.ActivationFunctionType
    OP = mybir.AluOpType
    DR = mybir.MatmulPerfMode.DoubleRow

    s_q = scal["s_q"]
    s_r = scal["s_r"]
    relu_scale = scal["relu_scale"]
    magic = scal["magic"]

    nc = bacc.Bacc(trn_type="TRN2", target_bir_lowering=False, debug=False)

    # Map all our activation funcs onto one table set so the table-load pass
    # doesn't ping-pong between sets (see baseline note).
    import types as _types
    from concourse.hw_specs import get_activation_tables as _gat
    import bass_rust as _bass_rust

    _OURS = {AF.Abs, AF.Relu, AF.Square, AF.Ln, AF.Exp, AF.Identity, AF.Copy,
             AF.Sign, AF.MemsetZero}

    def _patched_act_table_loads(self):
        has_activation = any(
            isinstance(i, mybir.InstActivation)
            for b in self.main_func.blocks
            for i in b.instructions
        )
        if not has_activation:
            return
        tables = []
        for name, fns in _gat(self.m.arch).items():
            if name != "natural_log_exp_and_others":
                fns = fns - _OURS
            tables.append((name, fns))
        _bass_rust.insert_act_table_loads(self, tables)

    nc.insert_act_table_loads = _types.MethodType(_patched_act_table_loads, nc)

    wq_d = nc.dram_tensor("wq", [P, 8, 2, P], fp8, kind="ExternalInput").ap()
    wrest_d = nc.dram_tensor("wrest", [P, WREST], u8, kind="ExternalInput").ap()
    chA_ds = [
        nc.dram_tensor(f"chA{c}", [P, 10, CS[c]], u8, kind="ExternalInput").ap()
        for c in range(NCH)
    ]
    chB_ds = [
        nc.dram_tensor(f"chB{c}", [P, 12 * CS[c]], u8, kind="ExternalInput").ap()
        for c in range(NCH)
    ]
    out_d = nc.dram_tensor("out_stats", [P, 2 * NCH], f32, kind="ExternalOutput").ap()

    with tile.TileContext(nc) as tc:
        with (
            tc.tile_pool(name="const", bufs=1) as cpool,
            tc.tile_pool(name="work", bufs=1) as wpool,
            tc.tile_pool(name="qps", bufs=2, space="PSUM") as qpool,
            tc.tile_pool(name="hps", bufs=2, space="PSUM") as hpool,
        ):
            # DMA issue order = earliest-needed first. Sync-engine triggers
            # serialize at ~676ns each, so chunk tensors are split q-side
            # (chA: dt8+gq8) vs M-side (chB: gm/am rows, needed 2 steps later).
            wq = cpool.tile_from(wq_d)
            chAs = [None] * NCH
            chBs = [None] * NCH
            chAs[0] = cpool.tile_from(chA_ds[0], name="chA0")
            wrest = cpool.tile_from(wrest_d)
            chAs[1] = cpool.tile_from(chA_ds[1], name="chA1")
            chAs[2] = cpool.tile_from(chA_ds[2], name="chA2")
            chBs[0] = cpool.tile_from(chB_ds[0], name="chB0")
            for c in range(3, NCH):
                chAs[c] = cpool.tile_from(chA_ds[c], name=f"chA{c}")
                chBs[c - 2] = cpool.tile_from(chB_ds[c - 2], name=f"chB{c - 2}")
            for c in range(NCH - 2, NCH):
                chBs[c] = cpool.tile_from(chB_ds[c], name=f"chB{c}")

            invpt = wrest[:, OFF_INVPT:OFF_A0INV].bitcast(fp8).rearrange(
                "p (kg nb i m) -> p kg nb i m", kg=4, nb=4, i=2, m=P
            )
            a0inv = wrest[:, OFF_A0INV:OFF_IDENT].bitcast(fp8).rearrange(
                "p (nb i m) -> p nb i m", nb=4, i=2, m=P
            )
            identb = wrest[:, OFF_IDENT:OFF_HSUPW].bitcast(bf16)
            hsupw = wrest[:, OFF_HSUPW:OFF_LNB].bitcast(bf16)
            lnbias = wrest[:, OFF_LNB:OFF_LNB + 4].bitcast(f32)

            stats = cpool.tile([P, 2 * NCH], f32, tag="stats")
            ones = cpool.tile([P, 512], bf16, tag="ones")
            nc.gpsimd.memset(ones, 1.0)

            qsbs, absqs, hl8s = {}, {}, {}

            # Dense PE warmup during the input-DMA wait: the HAM clock gate
            # needs a full ~3.4us busy window before releasing 2x clock, and
            # the fill phase otherwise runs entirely at K=4 (half speed).
            # ones is memset on-device, so this starts before any DMA lands.
            wtile0 = hpool.tile([P, 2, 512], f32, tag="hp", name="warm0")
            for wi in range(48):
                nc.tensor.matmul(
                    wtile0[:, wi % 2, 0:64],
                    ones[:, 0:P],
                    ones[:, 0:64],
                    start=True, stop=True,
                )

            def wt(name, shape, dt_, c, cross):
                # size-class tagged tile; 256-chunks (0 and 4) never overlap
                z = shape[-1]
                bufs = (2 if z == 512 else 1) if cross else 1
                return wpool.tile(shape, dt_, name=f"{name}{z}", tag=f"{name}{z}", bufs=bufs)

            for t in range(NCH + 2):
                c_a, c_b, c_d = t, t - 1, t - 2

                # ---- A: q matmuls (fp8 DR) + fused drain(descale)+gather-add
                if c_a < NCH:
                    z = CS[c_a]
                    chA = chAs[c_a]
                    dt8 = chA[:, 0:2, :].bitcast(fp8)
                    gq8 = chA[:, 2:10, :].bitcast(fp8)
                    qsb = wt("qsb", [P, 8, z], bf16, c_a, True)
                    absq = wt("absq", [P, 8, z], bf16, c_a, True)
                    for g in range(4):
                        qp = qpool.tile([P, 2, 512], f32, tag="qp")
                        for j in range(2):
                            nc.tensor.matmul(
                                qp[:, j, 0:z], wq[:, 2 * g + j], dt8,
                                start=True, stop=True, perf_mode=DR,
                            )
                        nc.vector.scalar_tensor_tensor(
                            qsb[:, 2 * g:2 * g + 2, :], qp[:, :, 0:z], s_q,
                            gq8[:, 2 * g:2 * g + 2, :], OP.mult, OP.add,
                        )
                    nc.vector.tensor_scalar(
                        absq.bitcast(i16), qsb.bitcast(i16),
                        0x7FFF, None, OP.bitwise_and,
                    )
                    qsbs[c_a] = qsb
                    absqs[c_a] = absq

                # ---- PE warmup: HAM clock-gate needs sustained activity or
                #      the whole fill phase runs at K=4 (half clock). Tiny
                #      N=64 matmuls pad the sparse fill bursts cheaply.
                if t in (0, 1, 2, 3):
                    wtile = hpool.tile([P, 2, 512], f32, tag="hp", name="warm")
                    for wi in range((12, 12, 32, 24)[t]):
                        nc.tensor.matmul(
                            wtile[:, wi % 2, 0:64],
                            ones[:, 0:P],
                            ones[:, 0:64],
                            start=True, stop=True,
                        )

                # ---- B: e = |q|^0.852 via Ln/Exp, hl8 = q*e (fp8 out)
                if 0 <= c_b < NCH:
                    z = CS[c_b]
                    lne = wt("lne", [P, 8, z], bf16, c_b, False)
                    e_t = wt("e_t", [P, 8, z], bf16, c_b, False)
                    hlb = wt("hlb", [P, 8, z], bf16, c_b, True)
                    nc.scalar.activation(
                        lne, absqs.pop(c_b), AF.Ln, bias=lnbias[:, 0:1]
                    )
                    nc.scalar.activation(e_t, lne, AF.Exp, scale=0.852)
                    nc.vector.tensor_tensor(hlb, qsbs.pop(c_b), e_t, OP.mult)
                    hl8s[c_b] = hlb

                # ---- CE: H matmuls (hsup-injected), relu, fused bit-sqrt,
                #      d_leak = gm*sq (negated) PE-injected into the residual
                #      PSUM, square+accumulate read straight from PSUM.
                if 0 <= c_d < NCH:
                    z = CS[c_d]
                    chA = chAs[c_d]
                    chB = chBs[c_d]
                    dt8 = chA[:, 0:2, :].bitcast(fp8)
                    gmb = chB[:, 0:8 * z].bitcast(bf16).rearrange(
                        "p (nb z) -> p nb z", nb=4, z=z
                    )
                    am8 = chB[:, 8 * z:12 * z].bitcast(fp8).rearrange(
                        "p (nb z) -> p nb z", nb=4, z=z
                    )
                    hl8 = hl8s.pop(c_d)
                    rl = wt("rl", [P, 4, z], bf16, c_d, True)
                    sq = wt("sq", [P, 4, z], bf16, c_d, True)
                    dl = wt("dl", [P, 4, z], bf16, c_d, True)
                    hps = []
                    for pr in range(2):
                        psl = slice(2 * pr, 2 * pr + 2)
                        hp = hpool.tile([P, 2, 512], f32, tag="hp", name="hp")
                        for j in range(2):
                            nb = 2 * pr + j
                            nc.tensor.matmul(
                                hp[:, j, 0:z],
                                hsupw[0:1, nb * P:(nb + 1) * P],
                                ones[0:1, 0:z],
                                start=True, stop=False,
                            )
                            for kc in range(8):
                                nc.tensor.matmul(
                                    hp[:, j, 0:z],
                                    invpt[:, kc >> 1, nb, kc & 1, :],
                                    hl8[:, kc, :],
                                    start=False, stop=(kc == 7),
                                )
                        nc.scalar.activation(
                            rl[:, psl, :], hp[:, :, 0:z], AF.Relu,
                            scale=relu_scale,
                        )
                        hps.append(hp)
                    rps = []
                    for pr in range(2):
                        rp = hpool.tile([P, 2, 512], f32, tag="hp", name="rp")
                        for j in range(2):
                            nb = 2 * pr + j
                            nc.tensor.matmul(
                                rp[:, j, 0:z], a0inv[:, nb], dt8,
                                start=True, stop=False, perf_mode=DR,
                            )
                            nc.tensor.matmul(
                                rp[:, j, 0:z], identb, am8[:, nb, :],
                                start=False, stop=False,
                            )
                        rps.append(rp)
                    for pr in range(2):
                        psl = slice(2 * pr, 2 * pr + 2)
                        nc.vector.tensor_scalar(
                            sq[:, psl, :].bitcast(i16), rl[:, psl, :].bitcast(i16),
                            1, None, OP.logical_shift_right,
                        )
                        nc.vector.tensor_scalar(
                            sq[:, psl, :].bitcast(i16), sq[:, psl, :].bitcast(i16),
                            magic, None, OP.add,
                        )
                        nc.vector.tensor_tensor(
                            dl[:, psl, :], gmb[:, psl, :], sq[:, psl, :], OP.mult
                        )
                    for pr in range(2):
                        rp = rps[pr]
                        for j in range(2):
                            nc.tensor.matmul(
                                rp[:, j, 0:z], identb, dl[:, 2 * pr + j, :],
                                start=False, stop=True,
                            )
                    for pr in range(2):
                        scr = wt("scr", [P, 2, z], bf16, c_d, False)
                        nc.scalar.activation(
                            scr, rps[pr][:, :, 0:z], AF.Square, scale=s_r,
                            accum_out=stats[:, 2 * c_d + pr:2 * c_d + pr + 1],
                        )

            nc.sync.dma_start(out_d, stats)

    nc.compile()
    return nc


def _host_prep(inputs):
    D = np.asarray(inputs["D"], np.float32)
    leak = np.asarray(inputs["leak_id"]).reshape(-1).astype(np.int64)
    A0 = np.asarray(inputs["A0"], np.float32)
    inv = np.asarray(inputs["inv"], np.float32)
    M = np.asarray(inputs["M"], np.float32)
    supply = np.asarray(inputs["supply"], np.float32)
    L = np.asarray(inputs["L"], np.float32)
    d = np.asarray(inputs["d"], np.float32)
    C = np.asarray(inputs["C"], np.float32)
    a = float(np.asarray(inputs["a"]))
    Cd = float(np.asarray(inputs["Cd"]))
    W1 = np.asarray(inputs["W1"], np.float32)
    b1 = np.asarray(inputs["b1"], np.float32)
    W2 = np.asarray(inputs["W2"], np.float32)
    b2 = np.asarray(inputs["b2"], np.float32)
    W3 = np.asarray(inputs["W3"], np.float32)
    b3 = np.asarray(inputs["b3"], np.float32)
    base = np.asarray(inputs["base"], np.float32)

    ids = np.arange(N_PIPES, dtype=np.float32)[:, None]
    h = np.tanh(ids @ W1 + b1)
    h = np.tanh(h @ W2 + b2)
    table = base + (h @ W3 + b3)[:, 0]

    K = 10.667 * C**-1.852 * d**-4.871 * L
    c0 = Cd * a * math.sqrt(2.0 * G_ACC)

    perm = np.concatenate([np.arange(0, N_NODES, 2), np.arange(1, N_NODES, 2)])
    Mp = M[perm]
    invp = inv[perm]
    A0p = A0[perm]
    inv_ev = invp[:N_DEM]
    PM = inv.T @ M
    AM = A0p @ PM
    A0invF = A0p @ inv_ev.T
    A0invF[:N_DEM] -= np.eye(N_DEM, dtype=np.float32)
    hsup = invp @ supply

    def e4(x):
        return np.clip(np.asarray(x, np.float32), -240.0, 240.0).astype(E4M3)

    def p2(mx):
        return float(2.0 ** np.floor(np.log2(200.0 / mx)))

    b_iv = p2(np.abs(inv_ev).max())
    WH = (invp * K[None, :]).T * 2.0 ** (-1.852 * SIG)     # [pipe, node]
    w_wh = p2(np.abs(WH).max())
    b2s = p2(np.abs(A0invF).max())
    g2 = p2(np.abs(AM * table[None, :]).max())
    jv = b2s * SD / g2

    scal = {
        "s_q": float(2.0**SIG / (SD * b_iv)),
        "s_r": float(1.0 / (SD * b2s)),
        "relu_scale": float(-1.0 / w_wh),
        "magic": 0x1FC0,
    }

    # gather tables (fp8 bytes, row = leak tap)
    T8 = e4((PM * table[None, :]).T * 2.0**SIG)            # [1024, 1024]
    G1b = ((-c0 * g2) * Mp).T.astype(BF16)                  # [1024, 512] bf16
    A28 = e4((AM * table[None, :]).T * g2)                  # [1024, 512]

    # wq: [128, 8, 2, 128] fp8: inv_ev * b_iv DR blocks
    Wq = e4(inv_ev * b_iv)                                  # [256, 1024]
    wq_l = np.ascontiguousarray(
        Wq.reshape(2, P, 8, P).transpose(1, 2, 0, 3)
    )

    # wrest packed u8
    wrest = np.zeros((P, WREST), np.uint8)
    WHs = e4(WH * w_wh)                                     # [1024, 512]
    invpt_l = WHs.reshape(4, 2, P, 4, P).transpose(2, 0, 3, 1, 4)
    wrest[:, OFF_INVPT:OFF_A0INV] = invpt_l.reshape(P, 4096).view(np.uint8)
    A8 = e4(A0invF.T * b2s)                                 # [256, 512]
    a0inv_l = A8.reshape(2, P, 4, P).transpose(1, 2, 0, 3)
    wrest[:, OFF_A0INV:OFF_IDENT] = a0inv_l.reshape(P, 1024).view(np.uint8)
    wrest[:, OFF_IDENT:OFF_HSUPW] = (
        (np.eye(P, dtype=np.float32) * jv).astype(BF16).view(np.uint8)
    )
    hsupw = np.zeros((P, N_NODES), BF16)
    hsupw[0] = (-hsup * w_wh).astype(BF16)
    wrest[:, OFF_HSUPW:OFF_LNB] = hsupw.view(np.uint8).reshape(P, 1024)
    lnb = np.full((P, 1), 1e-35, np.float32)
    wrest[:, OFF_LNB:OFF_LNB + 4] = lnb.view(np.uint8)

    D8 = e4(D * SD)                                         # [S, 256]

    per_core = []
    for cc in range(N_CORES):
        s0 = cc * SC
        lc = leak[s0:s0 + SC]
        m = {"wq": wq_l, "wrest": wrest}
        off = 0
        for c in range(NCH):
            z = CS[c]
            sl = slice(s0 + off, s0 + off + z)
            ll = lc[off:off + z]
            chA = np.empty((P, 10, z), np.uint8)
            chA[:, 0:2, :] = (
                D8[sl].T.reshape(2, P, z).transpose(1, 0, 2).view(np.uint8)
            )
            chA[:, 2:10, :] = (
                T8[ll].reshape(z, 8, P).transpose(2, 1, 0).view(np.uint8)
            )
            chB = np.empty((P, 12 * z), np.uint8)
            gm_l = np.ascontiguousarray(
                G1b[ll].reshape(z, 4, P).transpose(2, 1, 0)
            )
            chB[:, 0:8 * z] = gm_l.reshape(P, 4 * z).view(np.uint8)
            chB[:, 8 * z:12 * z] = (
                np.ascontiguousarray(
                    A28[ll].reshape(z, 4, P).transpose(2, 1, 0)
                ).reshape(P, 4 * z).view(np.uint8)
            )
            m[f"chA{c}"] = np.ascontiguousarray(chA)
            m[f"chB{c}"] = np.ascontiguousarray(chB)
            off += z
        per_core.append(m)
    return scal, per_core


LAST_RESULTS = None


def kernel(**inputs) -> np.ndarray:
    global LAST_RESULTS
    from concourse.bass_utils import run_bass_kernel_spmd

    scal, per_core = _host_prep(inputs)

    key = tuple(sorted(scal.items()))
    if _MODULE_CACHE.get("key") != key:
        _MODULE_CACHE["nc"] = _build_module(scal)
        _MODULE_CACHE["key"] = key
    nc = _MODULE_CACHE["nc"]

    import os

    res = run_bass_kernel_spmd(
        nc,
        per_core,
        core_ids=list(range(N_CORES)),
        trace=bool(os.environ.get("BASS_TRACE")),
    )
    LAST_RESULTS = res

    total = 0.0
    for r in res.results:
        total += float(r["out_stats"].astype(np.float64).sum())
    return np.float32(total / (S_TOTAL * N_NODES))


# revision 33
# speedup vs baseline: 1.0112x; 1.0112x over previous
"""Trainium2 Bass kernel for the water-network leak MSE model.

Math (reference):
    net(s)   = base[idx_s] + MLP(idx_s)                    (idx_s in [0,1024))
    q        = D @ inv_ev + net*PM[:, idx]                 (PM = inv^T M)
    hL       = K * q|q|^0.852,  K = 10.667 C^-1.852 d^-4.871 L
    H        = hsup - hL @ invp^T,  hsup = invp @ supply
    d_leak   = c0 * Mp[:, idx] * sqrt(relu(H)),  c0 = Cd*a*sqrt(2g)
    res      = D @ A0invF^T + net*AM[:, idx] - d_leak
    out      = mean(res^2)

Device strategy (8 cores data-parallel, 2048 samples/core, chunks
[256,512,512,512,256], software-pipelined A/B/CE stages 3 deep):
  The Hazen-Williams K is folded per-pipe into the H-matmul weights so q
  carries no per-pipe scale; everything feeding only d_leak tolerates
  fp8. The q matmuls (D8 @ inv_ev8, K=256) and the residual D-part
  (D8 @ A0invF8) run as fp8-e4m3 DoubleRow matmuls (2 K-blocks per
  instruction, ~1.4x PE throughput; D ships once, as fp8). H matmuls
  use fp8 weights with the bf16 hl stream (DR fp8-hl variants lose more
  on the DVE fp8 conversion, which drops to 1x rate, than DR wins on
  PE). hsup is PE-injected into the H PSUM via a K=1 matmul so relu
  needs no per-bank bias and runs on 2-bank pairs. d_leak is computed
  NEGATED (table -c0*Mp*g2, bit-trick sqrt (i>>1)+0x1FC0) and
  PE-injected into the residual PSUM through the same jv*identity used
  for the AM gather rows, so Square reads the residual straight from
  PSUM with a descale factor and accumulates per-chunk partial sums -
  no residual drain op at all. The q drain is a fused
  scalar_tensor_tensor (PSUM descale + fp8 gather-row add).
  Per-sample gather rows (PM*table fp8, c0*Mp bf16, AM*table fp8) are
  gathered on host; per chunk they ship as two tensors (q-side chA /
  M-side chB, needed 2 steps apart) and all weights as one packed
  tensor: 13 DMA triggers total (each costs ~0.7us serialized on the
  Sync engine - 19 triggers dominated the old 14us head). PSUM: qp
  [P,2,CH]x2 ring (stage A) + hp [P,2,CH]x2 ring shared by the H and
  residual accumulators (alloc order hp0,hp1,rp0,rp1 keeps reuse one
  step apart) = 8 banks. Tiny N=64 warmup matmuls pad the fill-phase
  bursts against the HAM half-clock gate.
  Measured: ~87us (baseline 87.2us), rel err 7.7e-4 (gate 2e-2),
  validated bit-exactly against a numpy emulation of every
  quantization/bit-trick choice.
"""

import math

import numpy as np
import ml_dtypes

P = 128
N_CORES = 8
S_TOTAL = 16384
SC = S_TOTAL // N_CORES
CS = [256, 512, 512, 512, 256]   # per-chunk sample counts
NCH = len(CS)
N_NODES = 512
N_PIPES = 1024
N_DEM = 256
G_ACC = 9.80665
SIG = 1
SD = 2.0 ** 7

BF16 = ml_dtypes.bfloat16
E4M3 = ml_dtypes.float8_e4m3fn

# wrest packing offsets (bytes per partition)
OFF_INVPT = 0          # [4,4,2,128] fp8        -> 4096
OFF_A0INV = 4096       # [4,2,128] fp8          -> 1024
OFF_IDENT = 5120       # [128] bf16             -> 256
OFF_HSUPW = 5376       # [512] bf16             -> 1024
OFF_LNB = 6400         # [1] f32                -> 4
WREST = 6416

_MODULE_CACHE: dict = {}


def _build_module(scal):
    import concourse.bacc as bacc
    import concourse.mybir as mybir
    import concourse.tile as tile

    f32 = mybir.dt.float32
    bf16 = mybir.dt.bfloat16
    fp8 = mybir.dt.float8e4
    u8 = mybir.dt.uint8
    i16 = mybir.dt.int16
    AF = mybir.ActivationFunctionType
    OP = mybir.AluOpType
    DR = mybir.MatmulPerfMode.DoubleRow

    s_q = scal["s_q"]
    s_r = scal["s_r"]
    relu_scale = scal["relu_scale"]
    magic = scal["magic"]

    nc = bacc.Bacc(trn_type="TRN2", target_bir_lowering=False, debug=False)

    # Map all our activation funcs onto one table set so the table-load pass
    # doesn't ping-pong between sets (see baseline note).
    import types as _types
    from concourse.hw_specs import get_activation_tables as _gat
    import bass_rust as _bass_rust

    _OURS = {AF.Abs, AF.Relu, AF.Square, AF.Ln, AF.Exp, AF.Identity, AF.Copy,
             AF.Sign, AF.MemsetZero}

    def _patched_act_table_loads(self):
        has_activation = any(
            isinstance(i, mybir.InstActivation)
            for b in self.main_func.blocks
            for i in b.instructions
        )
        if not has_activation:
            return
        tables = []
        for name, fns in _gat(self.m.arch).items():
            if name != "natural_log_exp_and_others":
                fns = fns - _OURS
            tables.append((name, fns))
        _bass_rust.insert_act_table_loads(self, tables)

    nc.insert_act_table_loads = _types.MethodType(_patched_act_table_loads, nc)

    wq_d = nc.dram_tensor("wq", [P, 8, 2, P], fp8, kind="ExternalInput").ap()
    wrest_d = nc.dram_tensor("wrest", [P, WREST], u8, kind="ExternalInput").ap()
    chA0a_d = nc.dram_tensor("chA0a", [P, 2, CS[0]], u8, kind="ExternalInput").ap()
    chA_ds = [
        nc.dram_tensor(
            f"chA{c}", [P, 8 if c == 0 else 10, CS[c]], u8,
            kind="ExternalInput",
        ).ap()
        for c in range(NCH)
    ]
    chB_ds = [
        nc.dram_tensor(f"chB{c}", [P, 12 * CS[c]], u8, kind="ExternalInput").ap()
        for c in range(NCH)
    ]
    out_d = nc.dram_tensor("out_stats", [P, 2 * NCH], f32, kind="ExternalOutput").ap()

    with tile.TileContext(nc) as tc:
        with (
            tc.tile_pool(name="const", bufs=1) as cpool,
            tc.tile_pool(name="work", bufs=1) as wpool,
            tc.tile_pool(name="qps", bufs=2, space="PSUM") as qpool,
            tc.tile_pool(name="hps", bufs=2, space="PSUM") as hpool,
        ):
            # DMA issue order = earliest-needed first. Sync-engine triggers
            # serialize at ~676ns each, so chunk tensors are split q-side
            # (chA: dt8+gq8) vs M-side (chB: gm/am rows, needed 2 steps later).
            wq = cpool.tile_from(wq_d)
            chAs = [None] * NCH
            chBs = [None] * NCH
            chA0a = cpool.tile_from(chA0a_d, name="chA0a")
            chAs[0] = cpool.tile_from(chA_ds[0], name="chA0")
            wrest = cpool.tile_from(wrest_d)
            chAs[1] = cpool.tile_from(chA_ds[1], name="chA1")
            chAs[2] = cpool.tile_from(chA_ds[2], name="chA2")
            chBs[0] = cpool.tile_from(chB_ds[0], name="chB0")
            for c in range(3, NCH):
                chAs[c] = cpool.tile_from(chA_ds[c], name=f"chA{c}")
                chBs[c - 2] = cpool.tile_from(chB_ds[c - 2], name=f"chB{c - 2}")
            for c in range(NCH - 2, NCH):
                chBs[c] = cpool.tile_from(chB_ds[c], name=f"chB{c}")

            invpt = wrest[:, OFF_INVPT:OFF_A0INV].bitcast(fp8).rearrange(
                "p (kg nb i m) -> p kg nb i m", kg=4, nb=4, i=2, m=P
            )
            a0inv = wrest[:, OFF_A0INV:OFF_IDENT].bitcast(fp8).rearrange(
                "p (nb i m) -> p nb i m", nb=4, i=2, m=P
            )
            identb = wrest[:, OFF_IDENT:OFF_HSUPW].bitcast(bf16)
            hsupw = wrest[:, OFF_HSUPW:OFF_LNB].bitcast(bf16)
            lnbias = wrest[:, OFF_LNB:OFF_LNB + 4].bitcast(f32)

            stats = cpool.tile([P, 2 * NCH], f32, tag="stats")
            ones = cpool.tile([P, 512], bf16, tag="ones")
            nc.gpsimd.memset(ones, 1.0)

            qsbs, absqs, hl8s = {}, {}, {}

            # Dense PE warmup during the input-DMA wait: the HAM clock gate
            # needs a full ~3.4us busy window before releasing 2x clock, and
            # the fill phase otherwise runs entirely at K=4 (half speed).
            # ones is memset on-device, so this starts before any DMA lands.
            wtile0 = hpool.tile([P, 2, 512], f32, tag="hp", name="warm0")
            for wi in range(48):
                nc.tensor.matmul(
                    wtile0[:, wi % 2, 0:64],
                    ones[:, 0:P],
                    ones[:, 0:64],
                    start=True, stop=True,
                )

            def wt(name, shape, dt_, c, cross):
                # size-class tagged tile; 256-chunks (0 and 4) never overlap
                z = shape[-1]
                bufs = (2 if z == 512 else 1) if cross else 1
                return wpool.tile(shape, dt_, name=f"{name}{z}", tag=f"{name}{z}", bufs=bufs)

            for t in range(NCH + 2):
                c_a, c_b, c_d = t, t - 1, t - 2

                # ---- A: q matmuls (fp8 DR) + fused drain(descale)+gather-add
                if c_a < NCH:
                    z = CS[c_a]
                    chA = chAs[c_a]
                    if c_a == 0:
                        dt8 = chA0a[:, :, :].bitcast(fp8)
                        gq8 = chA[:, 0:8, :].bitcast(fp8)
                    else:
                        dt8 = chA[:, 0:2, :].bitcast(fp8)
                        gq8 = chA[:, 2:10, :].bitcast(fp8)
                    qsb = wt("qsb", [P, 8, z], bf16, c_a, True)
                    absq = wt("absq", [P, 8, z], bf16, c_a, True)
                    for g in range(4):
                        qp = qpool.tile([P, 2, 512], f32, tag="qp")
                        for j in range(2):
                            nc.tensor.matmul(
                                qp[:, j, 0:z], wq[:, 2 * g + j], dt8,
                                start=True, stop=True, perf_mode=DR,
                            )
                        nc.vector.scalar_tensor_tensor(
                            qsb[:, 2 * g:2 * g + 2, :], qp[:, :, 0:z], s_q,
                            gq8[:, 2 * g:2 * g + 2, :], OP.mult, OP.add,
                        )
                    nc.vector.tensor_scalar(
                        absq.bitcast(i16), qsb.bitcast(i16),
                        0x7FFF, None, OP.bitwise_and,
                    )
                    qsbs[c_a] = qsb
                    absqs[c_a] = absq

                # ---- PE warmup: HAM clock-gate needs sustained activity or
                #      the whole fill phase runs at K=4 (half clock). Tiny
                #      N=64 matmuls pad the sparse fill bursts cheaply.
                if t in (0, 1, 2, 3):
                    wtile = hpool.tile([P, 2, 512], f32, tag="hp", name="warm")
                    for wi in range(12):
                        nc.tensor.matmul(
                            wtile[:, wi % 2, 0:64],
                            ones[:, 0:P],
                            ones[:, 0:64],
                            start=True, stop=True,
                        )

                # ---- B: e = |q|^0.852 via Ln/Exp, hl8 = q*e (fp8 out)
                if 0 <= c_b < NCH:
                    z = CS[c_b]
                    lne = wt("lne", [P, 8, z], bf16, c_b, False)
                    e_t = wt("e_t", [P, 8, z], bf16, c_b, False)
                    hlb = wt("hlb", [P, 8, z], bf16, c_b, True)
                    nc.scalar.activation(
                        lne, absqs.pop(c_b), AF.Ln, bias=lnbias[:, 0:1]
                    )
                    nc.scalar.activation(e_t, lne, AF.Exp, scale=0.852)
                    nc.vector.tensor_tensor(hlb, qsbs.pop(c_b), e_t, OP.mult)
                    hl8s[c_b] = hlb

                # ---- CE: H matmuls (hsup-injected), relu, fused bit-sqrt,
                #      d_leak = gm*sq (negated) PE-injected into the residual
                #      PSUM, square+accumulate read straight from PSUM.
                if 0 <= c_d < NCH:
                    z = CS[c_d]
                    chA = chAs[c_d]
                    chB = chBs[c_d]
                    if c_d == 0:
                        dt8 = chA0a[:, :, :].bitcast(fp8)
                    else:
                        dt8 = chA[:, 0:2, :].bitcast(fp8)
                    gmb = chB[:, 0:8 * z].bitcast(bf16).rearrange(
                        "p (nb z) -> p nb z", nb=4, z=z
                    )
                    am8 = chB[:, 8 * z:12 * z].bitcast(fp8).rearrange(
                        "p (nb z) -> p nb z", nb=4, z=z
                    )
                    hl8 = hl8s.pop(c_d)
                    rl = wt("rl", [P, 4, z], bf16, c_d, True)
                    sq = wt("sq", [P, 4, z], bf16, c_d, True)
                    dl = wt("dl", [P, 4, z], bf16, c_d, True)
                    hps = []
                    for pr in range(2):
                        psl = slice(2 * pr, 2 * pr + 2)
                        hp = hpool.tile([P, 2, 512], f32, tag="hp", name="hp")
                        for j in range(2):
                            nb = 2 * pr + j
                            nc.tensor.matmul(
                                hp[:, j, 0:z],
                                hsupw[0:1, nb * P:(nb + 1) * P],
                                ones[0:1, 0:z],
                                start=True, stop=False,
                            )
                            for kc in range(8):
                                nc.tensor.matmul(
                                    hp[:, j, 0:z],
                                    invpt[:, kc >> 1, nb, kc & 1, :],
                                    hl8[:, kc, :],
                                    start=False, stop=(kc == 7),
                                )
                        nc.scalar.activation(
                            rl[:, psl, :], hp[:, :, 0:z], AF.Relu,
                            scale=relu_scale,
                        )
                        hps.append(hp)
                    rps = []
                    for pr in range(2):
                        rp = hpool.tile([P, 2, 512], f32, tag="hp", name="rp")
                        for j in range(2):
                            nb = 2 * pr + j
                            nc.tensor.matmul(
                                rp[:, j, 0:z], a0inv[:, nb], dt8,
                                start=True, stop=False, perf_mode=DR,
                            )
                            nc.tensor.matmul(
                                rp[:, j, 0:z], identb, am8[:, nb, :],
                                start=False, stop=False,
                            )
                        rps.append(rp)
                    for pr in range(2):
                        psl = slice(2 * pr, 2 * pr + 2)
                        nc.vector.tensor_scalar(
                            sq[:, psl, :].bitcast(i16), rl[:, psl, :].bitcast(i16),
                            1, None, OP.logical_shift_right,
                        )
                        nc.vector.tensor_scalar(
                            sq[:, psl, :].bitcast(i16), sq[:, psl, :].bitcast(i16),
                            magic, None, OP.add,
                        )
                        nc.vector.tensor_tensor(
                            dl[:, psl, :], gmb[:, psl, :], sq[:, psl, :], OP.mult
                        )
                    for pr in range(2):
                        rp = rps[pr]
                        for j in range(2):
                            nc.tensor.matmul(
                                rp[:, j, 0:z], identb, dl[:, 2 * pr + j, :],
                                start=False, stop=True,
                            )
                    for pr in range(2):
                        scr = wt("scr", [P, 2, z], bf16, c_d, False)
                        nc.scalar.activation(
                            scr, rps[pr][:, :, 0:z], AF.Square, scale=s_r,
                            accum_out=stats[:, 2 * c_d + pr:2 * c_d + pr + 1],
                        )

            nc.sync.dma_start(out_d, stats)

    nc.compile()
    return nc


def _host_prep(inputs):
    D = np.asarray(inputs["D"], np.float32)
    leak = np.asarray(inputs["leak_id"]).reshape(-1).astype(np.int64)
    A0 = np.asarray(inputs["A0"], np.float32)
    inv = np.asarray(inputs["inv"], np.float32)
    M = np.asarray(inputs["M"], np.float32)
    supply = np.asarray(inputs["supply"], np.float32)
    L = np.asarray(inputs["L"], np.float32)
    d = np.asarray(inputs["d"], np.float32)
    C = np.asarray(inputs["C"], np.float32)
    a = float(np.asarray(inputs["a"]))
    Cd = float(np.asarray(inputs["Cd"]))
    W1 = np.asarray(inputs["W1"], np.float32)
    b1 = np.asarray(inputs["b1"], np.float32)
    W2 = np.asarray(inputs["W2"], np.float32)
    b2 = np.asarray(inputs["b2"], np.float32)
    W3 = np.asarray(inputs["W3"], np.float32)
    b3 = np.asarray(inputs["b3"], np.float32)
    base = np.asarray(inputs["base"], np.float32)

    ids = np.arange(N_PIPES, dtype=np.float32)[:, None]
    h = np.tanh(ids @ W1 + b1)
    h = np.tanh(h @ W2 + b2)
    table = base + (h @ W3 + b3)[:, 0]

    K = 10.667 * C**-1.852 * d**-4.871 * L
    c0 = Cd * a * math.sqrt(2.0 * G_ACC)

    perm = np.concatenate([np.arange(0, N_NODES, 2), np.arange(1, N_NODES, 2)])
    Mp = M[perm]
    invp = inv[perm]
    A0p = A0[perm]
    inv_ev = invp[:N_DEM]
    PM = inv.T @ M
    AM = A0p @ PM
    A0invF = A0p @ inv_ev.T
    A0invF[:N_DEM] -= np.eye(N_DEM, dtype=np.float32)
    hsup = invp @ supply

    def e4(x):
        return np.clip(np.asarray(x, np.float32), -240.0, 240.0).astype(E4M3)

    def p2(mx):
        return float(2.0 ** np.floor(np.log2(200.0 / mx)))

    b_iv = p2(np.abs(inv_ev).max())
    WH = (invp * K[None, :]).T * 2.0 ** (-1.852 * SIG)     # [pipe, node]
    w_wh = p2(np.abs(WH).max())
    b2s = p2(np.abs(A0invF).max())
    g2 = p2(np.abs(AM * table[None, :]).max())
    jv = b2s * SD / g2

    scal = {
        "s_q": float(2.0**SIG / (SD * b_iv)),
        "s_r": float(1.0 / (SD * b2s)),
        "relu_scale": float(-1.0 / w_wh),
        "magic": 0x1FC0,
    }

    # gather tables (fp8 bytes, row = leak tap)
    T8 = e4((PM * table[None, :]).T * 2.0**SIG)            # [1024, 1024]
    G1b = ((-c0 * g2) * Mp).T.astype(BF16)                  # [1024, 512] bf16
    A28 = e4((AM * table[None, :]).T * g2)                  # [1024, 512]

    # wq: [128, 8, 2, 128] fp8: inv_ev * b_iv DR blocks
    Wq = e4(inv_ev * b_iv)                                  # [256, 1024]
    wq_l = np.ascontiguousarray(
        Wq.reshape(2, P, 8, P).transpose(1, 2, 0, 3)
    )

    # wrest packed u8
    wrest = np.zeros((P, WREST), np.uint8)
    WHs = e4(WH * w_wh)                                     # [1024, 512]
    invpt_l = WHs.reshape(4, 2, P, 4, P).transpose(2, 0, 3, 1, 4)
    wrest[:, OFF_INVPT:OFF_A0INV] = invpt_l.reshape(P, 4096).view(np.uint8)
    A8 = e4(A0invF.T * b2s)                                 # [256, 512]
    a0inv_l = A8.reshape(2, P, 4, P).transpose(1, 2, 0, 3)
    wrest[:, OFF_A0INV:OFF_IDENT] = a0inv_l.reshape(P, 1024).view(np.uint8)
    wrest[:, OFF_IDENT:OFF_HSUPW] = (
        (np.eye(P, dtype=np.float32) * jv).astype(BF16).view(np.uint8)
    )
    hsupw = np.zeros((P, N_NODES), BF16)
    hsupw[0] = (-hsup * w_wh).astype(BF16)
    wrest[:, OFF_HSUPW:OFF_LNB] = hsupw.view(np.uint8).reshape(P, 1024)
    lnb = np.full((P, 1), 1e-35, np.float32)
    wrest[:, OFF_LNB:OFF_LNB + 4] = lnb.view(np.uint8)

    D8 = e4(D * SD)                                         # [S, 256]

    per_core = []
    for cc in range(N_CORES):
        s0 = cc * SC
        lc = leak[s0:s0 + SC]
        m = {"wq": wq_l, "wrest": wrest}
        off = 0
        for c in range(NCH):
            z = CS[c]
            sl = slice(s0 + off, s0 + off + z)
            ll = lc[off:off + z]
            dtb = np.ascontiguousarray(
                D8[sl].T.reshape(2, P, z).transpose(1, 0, 2)
            ).view(np.uint8)
            gqb = np.ascontiguousarray(
                T8[ll].reshape(z, 8, P).transpose(2, 1, 0)
            ).view(np.uint8)
            if c == 0:
                m["chA0a"] = dtb
                chA = gqb
            else:
                chA = np.empty((P, 10, z), np.uint8)
                chA[:, 0:2, :] = dtb
                chA[:, 2:10, :] = gqb
            chB = np.empty((P, 12 * z), np.uint8)
            gm_l = np.ascontiguousarray(
                G1b[ll].reshape(z, 4, P).transpose(2, 1, 0)
            )
            chB[:, 0:8 * z] = gm_l.reshape(P, 4 * z).view(np.uint8)
            chB[:, 8 * z:12 * z] = (
                np.ascontiguousarray(
                    A28[ll].reshape(z, 4, P).transpose(2, 1, 0)
                ).reshape(P, 4 * z).view(np.uint8)
            )
            m[f"chA{c}"] = np.ascontiguousarray(chA)
            m[f"chB{c}"] = np.ascontiguousarray(chB)
            off += z
        per_core.append(m)
    return scal, per_core


LAST_RESULTS = None


def kernel(**inputs) -> np.ndarray:
    global LAST_RESULTS
    from concourse.bass_utils import run_bass_kernel_spmd

    scal, per_core = _host_prep(inputs)

    key = tuple(sorted(scal.items()))
    if _MODULE_CACHE.get("key") != key:
        _MODULE_CACHE["nc"] = _build_module(scal)
        _MODULE_CACHE["key"] = key
    nc = _MODULE_CACHE["nc"]

    import os

    res = run_bass_kernel_spmd(
        nc,
        per_core,
        core_ids=list(range(N_CORES)),
        trace=bool(os.environ.get("BASS_TRACE")),
    )
    LAST_RESULTS = res

    total = 0.0
    for r in res.results:
        total += float(r["out_stats"].astype(np.float64).sum())
    return np.float32(total / (S_TOTAL * N_NODES))
